# revision 21
# baseline (speedup 1.0000x reference)
"""Neural CDE Trainium2 kernel, v2.

Strategy: pure data parallelism over batch B=128 -> 8 cores x 16 rows.
Per core, the T-1=1023-step RK4 scan is a fully unrolled sequential
chain. Layout: activations [feature_on_partition, batch_on_free].

v2 redesign vs baseline (3x shorter critical chain per RK4 stage):

1. ONE activation table. The stock insert_act_table_loads pass resolves
   Exp -> exp_and_others and Ln -> natural_log, inserting a 1283ns
   LoadActFuncSet before nearly every activation (~10us/step). A Bacc
   subclass re-runs the pass with {Exp,Ln,Relu,Identity,Copy} stripped
   from every table except natural_log_exp_and_others, so the loop runs
   on a single resident table (one hoisted load).

2. tanh via r = 1/(1+e^{2v}): Exp on ACT (same table), then clamp+1 and
   reciprocal_approx_fast on DVE. k = S - 2*G^T(r.*Z) as in the baseline.

3. The state never leaves PSUM/matmul-land. Kernel state is
   fy_t = fw0 @ y_t (the layer-1 preactivation). Since ycur_{j+1} =
   y + a_j k_j and layer 1 is linear, every stage input preactivation
   is accumulated directly in PSUM:
     p1[j+1] = I128@fy + a_j*rowsum(fw0) (x) S  +  sum_c PM_c @ rZ_j
   where PM_c[p,w] = -2a * fw0[w, 16c+p%16] folds fw0 @ G^T(-2a) into
   one precomputed stationary. The step update reuses the stage regions:
     fy' = (1/3)(r1 + 2 r2 + r3 - fy) + (1/6)S2 + (1/6)fw0@kneg_4
   with the r-bank combine done by 3 off-chain DVE ops (each reading at
   most ONE PSUM operand - walrus IBVF027) and the 1/3 folded into an
   i3 = I/3 seed matmul, so only stage 3 needs PA accumulation matmuls.
   Seeds and korr matmuls are issued off the critical chain; only the
   4 PM/PA matmuls after each rZ are on it. y_T is recovered at readout
   with a single pinv(fw0) matmul (fw0 is 128x64, cond ~ 5.8).

   Per-stage chain: ACT{Exp,Ln} -> PE{p2} -> ACT{Exp,Ln} -> PE{p3 x4}
   -> ACT{Exp} -> DVE{den,recip,rZ} -> PE{PM x4} -> next stage.

4. dX variants and the korr rows S are precomputed on host and streamed
   blockwise via double-buffered DMA; the 16x partition replication of Z
   happens in the DMA itself via a stride-0 access-pattern dim, so only
   ~1.6MB/core is uploaded.
"""

import numpy as np

B, T, D, H, W = 128, 1024, 8, 64, 128
NCORES = 8
BS = B // NCORES          # 16 batch rows per core
NSTEPS_FULL = T - 1       # 1023
TBLK = 96                 # steps per DMA block

_AJ = (0.5, 0.5, 1.0)          # stage input scale a_j for j=0,1,2
_UJ = (1.0 / 6.0, 1.0 / 3.0, 1.0 / 3.0, 1.0 / 6.0)  # y' weights
_SROW = (0, 1, 1, 2)           # dX variant per stage

# wconst free-dim layout: name -> (partitions, free_offset, free_len)
_L = {}
_off = 0
for _name, _p, _f in [
    ("fw1p", W, W), ("fw2p", W, 512),
    ("pm05", 128, 512), ("pm10", 128, 512),
    ("pa16", 128, 512), ("pa13", 128, 512),
    ("i128", 128, 128), ("i3", 128, 128), ("b3l", 4, 128), ("b3r", 4, 64),
    ("fw0rs", 1, 128), ("fw0p", H, W),
    ("iw0p", D, W), ("iw1p", W, W), ("iw2p", W, H),
    ("x0T", D, BS), ("pinvT", 128, H), ("lwT", H, 1),
    ("fb0", W, 1), ("fb1", W, 1),
    ("ib0", W, 1), ("ib1", W, 1), ("ib2", H, 1),
    ("lbneg", 1, 1),
]:
    _L[_name] = (_p, _off, _f)
    _off += _f
WCONST_F = _off


def _nblk(nsteps):
    return (nsteps + TBLK - 1) // TBLK


def build_bass(nsteps):
    import concourse.bass as bass
    import concourse.bacc as bacc
    import concourse.mybir as mybir
    from concourse import tile
    from concourse.hw_specs import get_activation_tables
    import bass_rust as _bass_rust

    f32 = mybir.dt.float32
    AF = mybir.ActivationFunctionType
    ALU = mybir.AluOpType

    LOOP_FUNCS = {AF.Exp, AF.Ln, AF.Relu, AF.Identity, AF.Copy}
    ONE_TABLE = "natural_log_exp_and_others"

    class BaccOneTable(bacc.Bacc):
        """Bacc whose act-table pass may only satisfy the loop's
        activation functions from ONE table, so the fixpoint hoists a
        single LoadActFuncSet instead of thrashing tables per-op.
        Table ids stay canonical (same list order/names)."""

        def insert_act_table_loads(self):
            has_activation = any(
                isinstance(i, mybir.InstActivation)
                for b in self.main_func.blocks
                for i in b.instructions
            )
            if not has_activation:
                return
            tables = []
            for name, fns in get_activation_tables(self.m.arch).items():
                if name != ONE_TABLE:
                    fns = fns - LOOP_FUNCS
                tables.append((name, fns))
            _bass_rust.insert_act_table_loads(self, tables)

    nc = BaccOneTable(None)

    nblk = _nblk(nsteps)
    wc_d = nc.declare_dram_parameter("wconst", [128, WCONST_F], f32, isOutput=False)
    z_d = nc.declare_dram_parameter("zdat", [8, nblk * TBLK * 48], f32, isOutput=False)
    s_d = nc.declare_dram_parameter("sdat", [1, nblk * TBLK * 64], f32, isOutput=False)
    out_d = nc.declare_dram_parameter("out", [1, BS], f32, isOutput=True)

    with tile.TileContext(nc) as tc:
        with (
            tc.tile_pool(name="const", bufs=1) as cpool,
            tc.tile_pool(name="zblk", bufs=2) as zpool,
            tc.tile_pool(name="sblk", bufs=2) as spool,
            tc.tile_pool(name="fysb", bufs=2) as fypool,
            tc.tile_pool(name="work", bufs=2) as wpool,
            tc.tile_pool(name="rwork", bufs=2) as rpool,
            tc.tile_pool(name="rz", bufs=2) as rzpool,
            tc.tile_pool(name="ps_fy", bufs=2, space="PSUM") as ps_fy,
            tc.tile_pool(name="ps_r1", bufs=1, space="PSUM") as ps_r1,
            tc.tile_pool(name="ps_r2", bufs=1, space="PSUM") as ps_r2,
            tc.tile_pool(name="ps_r3", bufs=1, space="PSUM") as ps_r3,
            tc.tile_pool(name="ps_p2", bufs=1, space="PSUM") as ps_p2,
            tc.tile_pool(name="ps_p3", bufs=1, space="PSUM") as ps_p3,
            tc.tile_pool(name="ps_sc", bufs=1, space="PSUM") as ps_sc,
        ):
            wc = cpool.tile([128, WCONST_F], f32, tag="wconst")
            nc.sync.dma_start(wc[:], wc_d[:])

            zt = {}
            st = {}

            def load_blk(b):
                if b >= nblk:
                    return
                zt[b] = zpool.tile([128, TBLK * 48], f32, tag="z", name="ztile")
                zsrc = z_d[:, b * TBLK * 48 : (b + 1) * TBLK * 48]
                zsrc = bass.AP(
                    zsrc.tensor, zsrc.offset, [zsrc.ap[0], [0, 16], zsrc.ap[1]]
                )
                nc.sync.dma_start(zt[b][:], zsrc)
                st[b] = spool.tile([1, TBLK * 64], f32, tag="s", name="stile")
                nc.sync.dma_start(st[b][:], s_d[:, b * TBLK * 64 : (b + 1) * TBLK * 64])

            load_blk(0)
            load_blk(1)

            def C(name):
                p, o, f = _L[name]
                return wc[0:p, o : o + f]

            # Warm non-PE engines' vector clocks on the first DMAs so hot
            # ops don't carry a DMA wait alongside an engine wait.
            warm = wpool.tile([1, 4], f32, tag="warm")
            nc.scalar.activation(warm[0:1, 0:1], wc[0:1, 0:1], AF.Copy)
            nc.vector.tensor_copy(warm[0:1, 1:2], wc[0:1, 0:1])
            nc.vector.tensor_copy(warm[0:1, 2:3], zt[0][0:1, 0:1])
            nc.vector.tensor_copy(warm[0:1, 3:4], st[0][0:1, 0:1])

            # ---- init MLP: y0 = relu-MLP(x0); FY_0 = fw0 @ y0 ----
            pi1 = ps_p3.tile([W, BS], f32, tag="p3")
            nc.tensor.matmul(pi1[:], C("iw0p"), C("x0T"), start=True, stop=True)
            h1 = wpool.tile([W, BS], f32, tag="h")
            nc.scalar.activation(h1[:], pi1[:], AF.Relu, bias=C("ib0"))
            pi2 = ps_p3.tile([W, BS], f32, tag="p3")
            nc.tensor.matmul(pi2[:], C("iw1p"), h1[:], start=True, stop=True)
            h2 = wpool.tile([W, BS], f32, tag="h")
            nc.scalar.activation(h2[:], pi2[:], AF.Relu, bias=C("ib1"))
            pk = ps_p2.tile([H, BS], f32, tag="p2")
            nc.tensor.matmul(pk[:], C("iw2p"), h2[:], start=True, stop=True)
            y0 = wpool.tile([H, BS], f32, tag="h")
            nc.scalar.activation(y0[:], pk[:], AF.Identity, bias=C("ib2"))

            fyb = ps_fy.tile([128, BS], f32, tag="fy")
            nc.tensor.matmul(fyb[:], C("fw0p"), y0[:], start=True, stop=True)

            # ---- the scan ----
            for t in range(nsteps):
                b = t // TBLK
                toff = (t - b * TBLK)
                if toff == 0 and b + 1 < nblk:
                    load_blk(b + 1)
                    nc.vector.tensor_copy(warm[0:1, 2:3], zt[b + 1][0:1, 0:1])
                zcur = zt[b]
                scur = st[b]
                zoff = toff * 48
                soff = toff * 64

                # FY_{t+1} bank: seeded mid-step from the r-bank combine
                fyb_next = ps_fy.tile([128, BS], f32, tag="fy")
                rgs = []
                for j, pool in enumerate((ps_r1, ps_r2, ps_r3)):
                    rg = pool.tile([128, BS], f32, tag="rg", name="rgtile")
                    rgs.append(rg)

                for j in range(4):
                    pin = fyb[:] if j == 0 else rgs[j - 1][:]

                    # p3 bank bias seed (off-chain: only WAR on prev stage)
                    p3b = ps_p3.tile([128, 4 * BS], f32, tag="p3")
                    nc.tensor.matmul(p3b[:], C("b3l"), C("b3r"), start=True, stop=False)

                    sc = ps_sc.tile([128, 32], f32, tag="sc")

                    # layer 1: softplus(p1) ; p1 = pin + fb0
                    nc.scalar.activation(sc[:, 0:16], pin, AF.Exp, bias=C("fb0"))

                    # Off-chain work that READS the bank u1 just consumed is
                    # emitted only now: the tile dep-tracker serializes
                    # same-bank readers in program order, so emitting these
                    # DVE ops earlier would put them on the chain ahead of u1.
                    if j == 0:
                        # fy_t -> SBUF, then seed the regions from it
                        fy_sb = fypool.tile([128, BS], f32, tag="fysb")
                        nc.vector.tensor_copy(fy_sb[:], fyb[:])
                        for i_, rg in enumerate(rgs):
                            nc.tensor.matmul(
                                rg[:], C("i128"), fy_sb[:], start=True, stop=False
                            )
                            so = soff + (i_ + 1) * BS
                            nc.tensor.matmul(
                                rg[:], C("fw0rs"), scur[0:1, so : so + BS],
                                start=False, stop=False,
                            )
                    elif j == 1:
                        acc1 = wpool.tile([128, BS], f32, tag="acc")
                        nc.vector.tensor_tensor(
                            acc1[:], rgs[0][:], fy_sb[:], ALU.subtract
                        )
                    elif j == 2:
                        acc12 = wpool.tile([128, BS], f32, tag="acc")
                        nc.vector.affine_then_add(
                            acc12[:], rgs[1][:], acc1[:], 2.0, 0.0
                        )
                    elif j == 3:
                        dcomb = wpool.tile([128, BS], f32, tag="acc")
                        nc.vector.affine_then_add(
                            dcomb[:], rgs[2][:], acc12[:], 1.0, 0.0
                        )
                        nc.tensor.matmul(
                            fyb_next[:], C("fw0rs"), scur[0:1, soff : soff + BS],
                            start=True, stop=False,
                        )

                    s1 = wpool.tile([W, BS], f32, tag="h")
                    nc.scalar.activation(s1[:], sc[:, 0:16], AF.Ln, bias=1.0)

                    # layer 2
                    p2b = ps_p2.tile([W, BS], f32, tag="p2")
                    nc.tensor.matmul(p2b[:], C("fw1p"), s1[:], start=True, stop=True)
                    nc.scalar.activation(sc[:, 16:32], p2b[:], AF.Exp, bias=C("fb1"))
                    s2 = wpool.tile([W, BS], f32, tag="h")
                    nc.scalar.activation(s2[:], sc[:, 16:32], AF.Ln, bias=1.0)

                    # layer 3 (4 chunks into the bias-seeded bank)
                    fw2p = C("fw2p")
                    for c in range(4):
                        nc.tensor.matmul(
                            p3b[:, c * BS : (c + 1) * BS],
                            fw2p[:, c * 128 : (c + 1) * 128],
                            s2[:],
                            start=False, stop=(c == 3),
                        )
                    if j == 3:
                        # i3@(r1+2r2+r3-fy) into the FY bank; emitted here so
                        # its DVE wait sits behind the p3 chunks on PE
                        nc.tensor.matmul(
                            fyb_next[:], C("i3"), dcomb[:], start=False, stop=False
                        )

                    # r = 1/(1+e^{2v}): Exp on ACT, clamp+1 and recip on DVE
                    texp = rpool.tile([128, 4 * BS], f32, tag="texp")
                    nc.scalar.activation(texp[:], p3b[:], AF.Exp, scale=2.0)
                    den = rpool.tile([128, 4 * BS], f32, tag="den")
                    nc.vector.tensor_scalar(
                        den[:], texp[:], 1.0e30, 1.0, ALU.min, ALU.add
                    )
                    r = rpool.tile([128, 4 * BS], f32, tag="r")
                    nc.vector.reciprocal_approx_fast(r[:], den[:])

                    # rZ = r .* Z_{s(j)}  (Z broadcast along the 4 chunks)
                    s_ = _SROW[j]
                    zsl = zcur[:, zoff + s_ * BS : zoff + (s_ + 1) * BS]
                    zb = bass.AP(
                        zsl.tensor, zsl.offset, [zsl.ap[0], [0, 4], zsl.ap[1]]
                    )
                    r3 = bass.AP(
                        r[:, :].tensor, r[:, :].offset,
                        [r[:, :].ap[0], [BS, 4], [1, BS]],
                    )
                    rz = rzpool.tile([128, 4, BS], f32, tag="rz")
                    nc.vector.tensor_tensor(rz[:], r3, zb, ALU.mult)

                    # chain: stage-input region for j+1 (PM mms)
                    if j < 3:
                        pm = C("pm05") if _AJ[j] == 0.5 else C("pm10")
                        for c in range(4):
                            nc.tensor.matmul(
                                rgs[j][:], pm[:, c * 128 : (c + 1) * 128], rz[:, c, :],
                                start=False, stop=(c == 3),
                            )
                    else:
                        # fy_{t+1} = i3@(r1 + 2 r2 + r3 - fy) + (1/6)S2
                        #           + (1/6)*fw0@kneg_4  (PA16 mms)
                        for c in range(4):
                            nc.tensor.matmul(
                                fyb_next[:], C("pa16")[:, c * 128 : (c + 1) * 128],
                                rz[:, c, :],
                                start=False, stop=(c == 3),
                            )


                fyb = fyb_next

            # ---- readout: sigmoid(lw @ pinv(fw0) @ fy_T + lb) ----
            fyT = fypool.tile([128, BS], f32, tag="fysb")
            nc.vector.tensor_copy(fyT[:], fyb[:])
            ytp = ps_p2.tile([H, BS], f32, tag="p2")
            nc.tensor.matmul(ytp[:], C("pinvT"), fyT[:], start=True, stop=True)
            yt = wpool.tile([H, BS], f32, tag="h")
            nc.scalar.activation(yt[:], ytp[:], AF.Copy)
            pr = ps_p3.tile([1, BS], f32, tag="p3")
            nc.tensor.matmul(pr[:], C("lwT"), yt[:], start=True, stop=True)
            er = wpool.tile([1, BS], f32, tag="warm2")
            nc.scalar.activation(er[:], pr[:], AF.Exp, bias=C("lbneg"), scale=-1.0)
            dr = wpool.tile([1, BS], f32, tag="warm2")
            nc.vector.tensor_scalar_add(dr[:], er[:], 1.0)
            rr = wpool.tile([1, BS], f32, tag="warm2")
            nc.vector.reciprocal(rr[:], dr[:])
            nc.sync.dma_start(out_d[:], rr[:])

    nc.compile()
    return nc


def prep_inputs(ts, coeff_d, coeff_c, coeff_b, coeff_a,
                iw0, ib0, iw1, ib1, iw2, ib2,
                fw0, fb0, fw1, fb1, fw2, fb2, lw, lb, nsteps=NSTEPS_FULL):
    """Build per-core input maps (host-side numpy prep)."""
    f = np.float32
    cd = np.asarray(coeff_d, f)[:, :nsteps, :]
    cc = np.asarray(coeff_c, f)[:, :nsteps, :]
    cb = np.asarray(coeff_b, f)[:, :nsteps, :]
    ca = np.asarray(coeff_a, f)

    # dX variants per RK4 stage (h == 1): s=0 @ t, s=1 @ t+1/2, s=2 @ t+1
    dX = [cb, 0.75 * cd + cc + cb, 3.0 * cd + 2.0 * cc + cb]  # [B,nsteps,D]

    fw0 = np.asarray(fw0, f)
    fw2 = np.asarray(fw2, f)
    fb2 = np.asarray(fb2, f)

    def fill(wcv, name, arr):
        p, o, fl = _L[name]
        assert arr.shape == (p, fl), (name, arr.shape, (p, fl))
        wcv[0:p, o : o + fl] = arr

    p_ar = np.arange(128)
    hmap = 16 * (np.arange(4)[:, None] // 1) + 0  # placeholder

    wc0 = np.zeros((128, WCONST_F), f)
    fill(wc0, "fw1p", np.ascontiguousarray(np.asarray(fw1, f).T))

    # fw2p[w, c*128 + p] = fw2[hd(c,p), w],  hd = (16c + p%16)*D + p//16
    fw2p = np.zeros((W, 512), f)
    b3l = np.zeros((4, 128), f)
    for c in range(4):
        h = 16 * c + (p_ar % 16)
        d = p_ar // 16
        hd = h * D + d
        fw2p[:, c * 128 + p_ar] = fw2[hd, :].T
        b3l[c, p_ar] = fb2[hd]
    fill(wc0, "fw2p", fw2p)
    fill(wc0, "b3l", b3l)
    b3r = np.zeros((4, 64), f)
    for c in range(4):
        b3r[c, c * BS : (c + 1) * BS] = 1.0
    fill(wc0, "b3r", b3r)

    # PM/PA[p, c*128 + w] = scal * fw0[w, 16c + p%16]
    def pmat(scal):
        m = np.zeros((128, 512), f)
        for c in range(4):
            h = 16 * c + (p_ar % 16)
            m[p_ar[:, None], c * 128 + np.arange(W)[None, :]] = scal * fw0[:, h].T
        return m

    fill(wc0, "pm05", pmat(-2.0 * 0.5))
    fill(wc0, "pm10", pmat(-2.0 * 1.0))
    fill(wc0, "pa16", pmat(-2.0 / 6.0))
    fill(wc0, "pa13", pmat(-2.0 / 3.0))
    fill(wc0, "i128", np.eye(128, dtype=f))
    fill(wc0, "i3", (np.eye(128) / 3.0).astype(f))
    fill(wc0, "fw0rs", fw0.sum(axis=1)[None, :])
    fill(wc0, "fw0p", np.ascontiguousarray(fw0.T))
    fill(wc0, "iw0p", np.ascontiguousarray(np.asarray(iw0, f).T))
    fill(wc0, "iw1p", np.ascontiguousarray(np.asarray(iw1, f).T))
    fill(wc0, "iw2p", np.ascontiguousarray(np.asarray(iw2, f).T))
    pinv = np.linalg.pinv(fw0.astype(np.float64)).astype(f)  # [H, 128]
    fill(wc0, "pinvT", np.ascontiguousarray(pinv.T))
    fill(wc0, "lwT", np.ascontiguousarray(np.asarray(lw, f).reshape(1, H).T))
    fill(wc0, "fb0", np.asarray(fb0, f)[:, None])
    fill(wc0, "fb1", np.asarray(fb1, f)[:, None])
    fill(wc0, "ib0", np.asarray(ib0, f)[:, None])
    fill(wc0, "ib1", np.asarray(ib1, f)[:, None])
    fill(wc0, "ib2", np.asarray(ib2, f)[:, None])
    fill(wc0, "lbneg", -np.asarray(lb, f).reshape(1, 1))

    nblk = _nblk(nsteps)
    npad = nblk * TBLK

    # korr rows: S_s[b,t] = sum_d dX_s ; sdat row r at step t:
    #   r=0: KS = sum_j u_j S_{s(j)} ; r=1..3: a_j * S_{s(j)}
    S = [d_.sum(axis=2) for d_ in dX]  # [B, nsteps]
    rows = [(1.0 / 6.0) * S[2], 0.5 * S[0], 0.5 * S[1], 1.0 * S[1]]

    in_maps = []
    for i in range(NCORES):
        sl = slice(i * BS, (i + 1) * BS)
        wcv = wc0.copy()
        fill(wcv, "x0T", np.ascontiguousarray(ca[sl, 0, :].T))

        # zdat[d, t*48 + s*16 + b] = dX_s[b, t, d]; DMA replicates 16x
        # over partitions (p//16 = d) via a stride-0 AP dim.
        z = np.zeros((8, npad, 3, BS), f)
        for s_ in range(3):
            z[:, :nsteps, s_, :] = dX[s_][sl].transpose(2, 1, 0)
        z = np.ascontiguousarray(z.reshape(8, npad * 48))

        sd = np.zeros((npad, 4, BS), f)
        for r_ in range(4):
            sd[:nsteps, r_, :] = rows[r_][sl].T
        sd = np.ascontiguousarray(sd.reshape(1, npad * 64))

        in_maps.append({"wconst": wcv, "zdat": z, "sdat": sd})
    return in_maps


_CACHE = {}
_EXEC_CACHE = {}


def _get_nc(nsteps):
    if nsteps not in _CACHE:
        _CACHE[nsteps] = build_bass(nsteps)
    return _CACHE[nsteps]


def _get_executor(nsteps):
    """Per-module cached runner. run_bass_kernel_spmd rebuilds its jit
    closure every call (~8s of retrace under axon); build the sharded
    executable once and reuse it for warm kernel() calls."""
    if nsteps in _EXEC_CACHE:
        return _EXEC_CACHE[nsteps]
    nc = _get_nc(nsteps)
    from concourse import bass_utils

    if not bass_utils.axon_active():
        def run(in_maps):
            res = bass_utils.run_bass_kernel_spmd(nc, in_maps, list(range(NCORES)))
            return [res.results[i]["out"] for i in range(NCORES)]

        _EXEC_CACHE[nsteps] = run
        return run

    import jax
    import concourse.mybir as mybir
    from jax.sharding import Mesh, PartitionSpec
    from jax.experimental.shard_map import shard_map
    from concourse import bass2jax

    bass2jax.install_neuronx_cc_hook()
    partition_name = nc.partition_id_tensor.name if nc.partition_id_tensor else None
    in_names, out_names, out_avals, zero_outs = [], [], [], []
    for alloc in nc.m.functions[0].allocations:
        if not isinstance(alloc, mybir.MemoryLocationSet):
            continue
        name = alloc.memorylocations[0].name
        if alloc.kind == "ExternalInput":
            if name != partition_name:
                in_names.append(name)
        elif alloc.kind == "ExternalOutput":
            shape = tuple(alloc.tensor_shape)
            dtype = mybir.dt.np(alloc.dtype)
            out_names.append(name)
            out_avals.append(jax.core.ShapedArray(shape, dtype))
            zero_outs.append(np.zeros(shape, dtype))
    n_params = len(in_names)
    n_outs = len(out_avals)
    all_in = list(in_names) + list(out_names)
    if partition_name is not None:
        all_in.append(partition_name)
    donate = tuple(range(n_params, n_params + n_outs))

    def _body(*args):
        operands = list(args)
        if partition_name is not None:
            operands.append(bass2jax.partition_id_tensor())
        outs = bass2jax._bass_exec_p.bind(
            *operands,
            out_avals=tuple(out_avals),
            in_names=tuple(all_in),
            out_names=tuple(out_names),
            lowering_input_output_aliases=(),
            sim_require_finite=True,
            sim_require_nnan=True,
            nc=nc,
        )
        return tuple(outs)

    devices = jax.devices()[:NCORES]
    mesh = Mesh(np.asarray(devices), ("core",))
    in_specs = (PartitionSpec("core"),) * (n_params + n_outs)
    out_specs = (PartitionSpec("core"),) * len(out_names)
    sharded = jax.jit(
        shard_map(_body, mesh=mesh, in_specs=in_specs, out_specs=out_specs,
                  check_rep=False),
        donate_argnums=donate,
        keep_unused=True,
    )

    def run(in_maps):
        concat_in = [
            np.concatenate([np.asarray(in_maps[c][nm]) for c in range(NCORES)], 0)
            for nm in in_names
        ]
        concat_zeros = [
            np.zeros((NCORES * z.shape[0], *z.shape[1:]), z.dtype)
            for z in zero_outs
        ]
        out_arrs = sharded(*concat_in, *concat_zeros)
        oi = out_names.index("out")
        full = np.asarray(out_arrs[oi]).reshape(NCORES, *out_avals[oi].shape)
        return [full[c] for c in range(NCORES)]

    _EXEC_CACHE[nsteps] = run
    return run


def kernel(**inputs):
    nsteps = NSTEPS_FULL
    in_maps = prep_inputs(nsteps=nsteps, **inputs)
    run = _get_executor(nsteps)
    outs = run(in_maps)
    outs = [outs[i].reshape(BS) for i in range(NCORES)]
    return np.concatenate(outs, axis=0).astype(np.float32)
